# revision 21
# baseline (speedup 1.0000x reference)
"""GPT-2 style causal self-attention block on 8 Trainium2 NeuronCores.

Problem: x[4,2048,768] -> qkv = x@c_attn_w + b -> 12-head causal attention
-> a@c_proj_w + b.  Sharding: batch (4) x head-group (2x6 heads) = 8 cores.
Each core computes its batch's qkv columns for its 6 heads, runs attention
for those heads over the full sequence, and produces a partial c_proj
output (contraction over its 384 of 768 a-dims).  The two partials per
batch are summed on the host (+ c_proj bias).

Per-core layouts (matmul operands bf16 except where noted, f32 PSUM):
  qT,kT  [384, 2048]  head-dim on partitions (3 chunks of 128 = head pairs;
                      the two heads of a chunk run as K=64 row-packed
                      matmuls via tile_position 0/64).  qkf8: produced by
                      fp8e4m3 DoubleRow matmuls (x*16, w*64 host-scaled,
                      1/1024 descale in the PSUM->SBUF copy); measured 2x
                      PE rate vs bf16 at equal K.
  V      [2048, 390]  rows on partitions, per head 64 v-cols + ones col
  S^T    [128k, 512q] psum blocks, fully-masked left columns skipped; exp on
                      ACT (scale=1/8 folded in); causal diagonal fixed
                      post-exp by a 0/1 tri-mask multiply (DVE bf16 2x)
  A.V    avr: out[q,v] orientation - matmul(lhsT=pt[128k,128q],
                      rhs=vt[128k,65]) accumulating [128q, 4*(64v+den)] over
                      k-blocks.  N=65 matmuls measured N-bound (26.7ns) on
                      HW - weight loads fully overlap, so this halves A.V
                      PE time vs the old [65,512] orientation, and the
                      denominator lands as a per-partition scalar.
  norm   reciprocal of den column -> tensor_scalar per qb -> bf16 a_n;
         PE-transpose (identity matmul) back to aT[384, 2048] for c_proj.
  out^T  c_proj partials accumulated per 128-q block, staged via SBUF.

Emission is J-major across pairs (attention for one 512-query superblock on
all 6 heads back-to-back, with next-superblock QKV tiles, V rows woven
between blocks as PE filler; all c_proj deferred into the ACT-bound final
superblock phase).

The rep body is software-pipelined ACROSS loop iterations: the J=3 phase
prefetches the NEXT rep's J=0 q/k/v tiles (k-half of pair p placed after
block (p,1) = last reader of kT[p][0]; v rows after the final block), and
first-rep warmup runs once outside the For_i. This removed the serial
warmup from the steady-state critical path: HW rep-slope went 248us
(baseline) -> ~142us. PSUM: s_ps 2x2 banks, mm_ps 1, o_ps 2, t_ps 1 = 8.
mm_ps=1 is compensated by splitting q/k filler halves into separate block
slots so the single buffer never head-of-line blocks the PE queue.
Measured on HW: fp8 DoubleRow = 2.0x bf16 at equal K; N=65 matmuls are
N-bound (weight loads fully overlap); exp on ACT is the bottleneck engine
(~122us busy in the cost model, ~92% occupied in the steady window).
"""

import numpy as np
import ml_dtypes

B, S, D = 4, 2048, 768
NH, DH = 12, 64
NCORES = 8
HPC = 6          # heads per core
PAIRS = 3        # head pairs per core
NQ = S // 512    # q superblocks
NKB = S // 128   # k blocks
BF16 = ml_dtypes.bfloat16
F8E4 = ml_dtypes.float8_e4m3
XS, WS = 16.0, 64.0          # fp8 input/weight scales for qk-gen

_COMPILED = {}


def _build_program(reps=1, avr=True, qkf8=True, mmb=None, ob=None):
    import contextlib
    import concourse.mybir as mybir
    import concourse.tile as tile
    from concourse import bacc, masks

    F32, B16, F8 = mybir.dt.float32, mybir.dt.bfloat16, mybir.dt.float8e4
    EXP = mybir.ActivationFunctionType.Exp
    ADD, MULT = mybir.AluOpType.add, mybir.AluOpType.mult
    DR = mybir.MatmulPerfMode.DoubleRow

    nc = bacc.Bacc(None, target_bir_lowering=False, debug=False)
    xt_d = nc.dram_tensor("xt", [D, S], B16, kind="ExternalInput")
    wqk_d = nc.dram_tensor("wqk", [D, 768], B16, kind="ExternalInput")
    wqkb_d = nc.dram_tensor("wqkb", [128, 6], F32, kind="ExternalInput")
    wva_d = nc.dram_tensor("wva", [D + 1, HPC * 65], B16, kind="ExternalInput")
    wp_d = nc.dram_tensor("wp", [PAIRS * 128, D], B16, kind="ExternalInput")
    mask_d = nc.dram_tensor("mask", [128, 128], B16, kind="ExternalInput")
    vbb_d = nc.dram_tensor("vbb", [128, HPC * 65], B16, kind="ExternalInput")
    if qkf8:
        xtf8_d = nc.dram_tensor("xtf8", [D, S], F8, kind="ExternalInput")
        wqkf8_d = nc.dram_tensor("wqkf8", [D, 768], F8, kind="ExternalInput")
    out_d = nc.dram_tensor("out", [S, D], F32, kind="ExternalOutput")

    KC = D // 128  # 6 contraction chunks

    with tile.TileContext(nc) as tc:
        with (
            tc.tile_pool(name="const", bufs=1) as cst,
            tc.tile_pool(name="acts", bufs=1) as acts,
            tc.tile_pool(name="pt", bufs=8) as ptp,
            tc.tile_pool(name="nrm", bufs=4) as nrm,
            tc.tile_pool(name="s_ps", bufs=2, space="PSUM") as s_ps,
            tc.tile_pool(name="mm_ps", bufs=(mmb or (1 if avr else 2)),
                         space="PSUM") as mm_ps,
            tc.tile_pool(name="o_ps", bufs=(ob or 2), space="PSUM") as o_ps,
            tc.tile_pool(name="t_ps", bufs=1, space="PSUM") as t_ps,
        ):
            xt = cst.tile([128, KC, S], B16, tag="xt", name="xt")
            ones = cst.tile([1, S], B16, tag="ones", name="ones")
            wqk = cst.tile([128, KC, 768], B16, tag="wqk", name="wqk")
            wqkb = cst.tile([128, 6], F32, tag="wqkb", name="wqkb")
            wva = cst.tile([128, KC, HPC * 65], B16, tag="wva", name="wva")
            wvab = cst.tile([1, HPC * 65], B16, tag="wvab", name="wvab")
            wp = cst.tile([128, PAIRS, D], B16, tag="wp", name="wp")
            mask = cst.tile([128, 128], B16, tag="mask", name="mask")
            vbb = cst.tile([128, HPC * 65], B16, tag="vbb", name="vbb")
            if qkf8:
                xtf8 = cst.tile([128, KC, S], F8, tag="xtf8", name="xtf8")
                wqkf8 = cst.tile([128, KC, 768], F8, tag="wqkf8", name="wqkf8")
            if avr:
                ident = cst.tile([128, 128], B16, tag="ident", name="ident")
                masks.make_identity(nc, ident[:])

            # DMAs ordered by first use
            if qkf8:
                for c in range(KC):
                    nc.sync.dma_start(wqkf8[:, c, :], wqkf8_d[128 * c:128 * c + 128, :])
                for n in range(NQ):
                    for c in range(KC):
                        nc.sync.dma_start(
                            xtf8[:, c, 512 * n:512 * n + 512],
                            xtf8_d[128 * c:128 * c + 128, 512 * n:512 * n + 512])
            else:
                for c in range(KC):
                    nc.sync.dma_start(wqk[:, c, :], wqk_d[128 * c:128 * c + 128, :])
            for n in range(NQ):
                for c in range(KC):
                    nc.sync.dma_start(
                        xt[:, c, 512 * n:512 * n + 512],
                        xt_d[128 * c:128 * c + 128, 512 * n:512 * n + 512])
            nc.sync.dma_start(wqkb[:], wqkb_d[:])
            nc.sync.dma_start(mask[:], mask_d[:])
            nc.sync.dma_start(vbb[:], vbb_d[:])
            for c in range(KC):
                nc.sync.dma_start(wva[:, c, :], wva_d[128 * c:128 * c + 128, :])
            nc.sync.dma_start(wvab[:], wva_d[D:D + 1])
            for c in range(PAIRS):
                nc.sync.dma_start(wp[:, c, :], wp_d[128 * c:128 * c + 128, :])
            nc.vector.memset(ones[:], 1.0)

            qT = [[acts.tile([128, 512], B16, tag=f"qT{p}_{n}", name=f"qT{p}_{n}")
                   for n in range(NQ)] for p in range(PAIRS)]
            kT = [[acts.tile([128, 512], B16, tag=f"kT{p}_{n}", name=f"kT{p}_{n}")
                   for n in range(NQ)] for p in range(PAIRS)]
            vt = [acts.tile([128, HPC * 65], B16, tag=f"v{r}", name=f"v{r}") for r in range(NKB)]
            aT = [[acts.tile([128, 512], B16, tag=f"aT{p}_{n}", name=f"aT{p}_{n}")
                   for n in range(NQ)] for p in range(PAIRS)]

            def small_ps():
                return mm_ps.tile([128, 512], F32, tag="mm", name="mm")

            def emit_qk_half(p, n, half):
                dst, m = ((qT[p][n], p), (kT[p][n], PAIRS + p))[half]
                ps = small_ps()
                if qkf8:
                    for c in range(0, KC, 2):
                        nc.tensor.matmul(
                            ps[:],
                            wqkf8[:, c:c + 2, 128 * m:128 * m + 128],
                            xtf8[:, c:c + 2, 512 * n:512 * n + 512],
                            start=(c == 0), stop=(c == KC - 2),
                            perf_mode=DR,
                        )
                    nc.vector.tensor_scalar(
                        out=dst[:], in0=ps[:], scalar1=1.0 / (XS * WS),
                        scalar2=wqkb[:, m:m + 1], op0=MULT, op1=ADD)
                else:
                    for c in range(KC):
                        nc.tensor.matmul(
                            ps[:],
                            wqk[:, c, 128 * m:128 * m + 128],
                            xt[:, c, 512 * n:512 * n + 512],
                            start=(c == 0), stop=(c == KC - 1),
                        )
                    nc.vector.tensor_scalar_add(dst[:], ps[:], wqkb[:, m:m + 1])

            def emit_qk_n(p, n):
                for half in (0, 1):
                    emit_qk_half(p, n, half)

            def emit_v(rows):
                for r in rows:
                    ps = small_ps()
                    pv = ps[:, 0:HPC * 65]
                    for c in range(KC):
                        nc.tensor.matmul(
                            pv, xt[:, c, 128 * r:128 * r + 128], wva[:, c, :],
                            start=(c == 0), stop=(c == KC - 1))
                    # bias + ones column folded in via broadcast tile
                    nc.vector.tensor_tensor(
                        out=vt[r][:], in0=pv, in1=vbb[:], op=ADD)

            def emit_head_J(p, hh, J):
                """One (head, q-superblock): S^T blocks, exp, A.V, normalize."""
                h = 2 * p + hh
                pb = 64 * hh  # partition base of this head in its pair chunk
                nkb = 4 * J + 4
                if avr:
                    ps_o = o_ps.tile([128, 260], F32, tag="o", name="o")
                else:
                    ps_o = o_ps.tile([128, 512], F32, tag="o", name="o")

                groups = [list(range(g, min(g + 2, nkb))) for g in range(0, nkb, 2)]
                stage = []  # (kbs, ps_s, window_start)

                def s_group(kbs):
                    ps_s = s_ps.tile([128, 1024], F32, tag="s", name="s")
                    for i, kb in enumerate(kbs):
                        o = max(kb - 4 * J, 0)  # skip fully-masked left columns
                        nc.tensor.matmul(
                            ps_s[:, 512 * i + 128 * o:512 * i + 512],
                            kT[p][kb // 4][pb:pb + 64,
                                           128 * (kb % 4):128 * (kb % 4) + 128],
                            qT[p][J][pb:pb + 64, 128 * o:],
                            start=True, stop=True,
                            tile_position=(pb, 0),
                        )
                    return ps_s, 0

                def av_group(kbs, ps_s, w0):
                    pt = ptp.tile([128, 1024], B16, tag="pt", name="pt")
                    # exp: one call over contiguous valid region when no gaps,
                    # else exact per-kb windows (diagonal groups)
                    offs = [max(kb - 4 * J, 0) * 128 for kb in kbs]
                    if all(o == 0 for o in offs):
                        nc.scalar.activation(pt[:, 0:512 * len(kbs)],
                                             ps_s[:, 0:512 * len(kbs)],
                                             EXP, scale=0.125)
                    else:
                        for i, o in enumerate(offs):
                            nc.scalar.activation(
                                pt[:, 512 * i + o:512 * i + 512],
                                ps_s[:, 512 * i + o:512 * i + 512],
                                EXP, scale=0.125)
                    for i, kb in enumerate(kbs):
                        o = kb - 4 * J
                        if o >= 0:  # causal 0/1 mask applied post-exp (bf16 2x)
                            d_sl = slice(512 * i + 128 * o, 512 * i + 128 * o + 128)
                            nc.vector.tensor_tensor(
                                out=pt[:, d_sl], in0=pt[:, d_sl], in1=mask[:],
                                op=MULT)
                        if o > 0:
                            nc.gpsimd.memset(pt[:, 512 * i:512 * i + 128 * o], 0.0)
                        if avr:
                            # out[q, v]: lhsT = attention weights (128k x 128q),
                            # rhs = V block (128k x 64v+den). N=65 runs N-bound.
                            # All 4 qb slices share one psum bank, so they form
                            # ONE accumulation group (2KB zero-region rule):
                            # start on the first matmul, stop on the last.
                            for qb in range(4):
                                if o > qb:
                                    continue  # fully-masked: pt block is zero
                                nc.tensor.matmul(
                                    ps_o[:, 65 * qb:65 * qb + 65],
                                    pt[:, 512 * i + 128 * qb:512 * i + 128 * qb + 128],
                                    vt[kb][:, 65 * h:65 * h + 65],
                                    start=(kb == 0 and qb == 0),
                                    stop=(kb == nkb - 1 and qb == 3),
                                    skip_group_check=True,
                                )
                        else:
                            nc.tensor.matmul(
                                ps_o[0:65, :],
                                vt[kb][:, 65 * h:65 * h + 65],
                                pt[:, 512 * i:512 * i + 512],
                                start=(kb == 0), stop=(kb == nkb - 1),
                            )

                # software-pipelined emission: S(g+1) before A.V(g)
                stage.append((groups[0], *s_group(groups[0])))
                for g in range(len(groups)):
                    if g + 1 < len(groups):
                        stage.append((groups[g + 1], *s_group(groups[g + 1])))
                    av_group(*stage[g])

                if avr:
                    # per-query denominators are a free-dim column: reciprocal
                    # on a [128, 4] strided view, then per-qb scalar multiply.
                    den_s = nrm.tile([128, 4], F32, tag="den_s", name="den_s")
                    dview = ps_o[:].rearrange("p (q c) -> p q c", c=65)[:, :, 64]
                    nc.vector.tensor_copy(den_s[:], dview)
                    rden = nrm.tile([128, 4], F32, tag="rden", name="rden")
                    nc.vector.reciprocal_approx_fast(out=rden[:], in_=den_s[:])
                    a_n = nrm.tile([128, 256], B16, tag="a_n", name="a_n")
                    for qb in range(4):
                        nc.vector.tensor_scalar_mul(
                            a_n[:, 64 * qb:64 * qb + 64],
                            ps_o[:, 65 * qb:65 * qb + 64],
                            rden[:, qb:qb + 1])
                    psT = t_ps.tile([64, 512], B16, tag="psT", name="psT")
                    for qb in range(4):
                        nc.tensor.transpose(
                            psT[:, 128 * qb:128 * qb + 128],
                            a_n[:, 64 * qb:64 * qb + 64],
                            ident[:])
                    nc.vector.tensor_copy(aT[p][J][pb:pb + 64, :], psT[:])
                else:
                    den = nrm.tile([1, 512], F32, tag="den", name="den")
                    nc.vector.tensor_copy(den[:], ps_o[64:65, :])
                    rden = nrm.tile([1, 512], F32, tag="rden", name="rden")
                    nc.vector.reciprocal_approx_fast(out=rden[:], in_=den[:])
                    rbc = nrm.tile([64, 512], F32, tag="rbc", name="rbc")
                    nc.gpsimd.partition_broadcast(rbc[:], rden[:], channels=64)
                    nc.vector.tensor_tensor(
                        out=aT[p][J][pb:pb + 64, :], in0=ps_o[0:64, :], in1=rbc[:],
                        op=MULT)

            def emit_cproj(qbs):
                for qb in qbs:
                    osb = nrm.tile([128, D], F32, tag="osb", name="osb")
                    for nb in range(2):
                        ps = small_ps()
                        pc = ps[:, 0:384]
                        for c in range(PAIRS):
                            nc.tensor.matmul(
                                pc,
                                aT[c][qb // 4][:, 128 * (qb % 4):128 * (qb % 4) + 128],
                                wp[:, c, 384 * nb:384 * nb + 384],
                                start=(c == 0), stop=(c == PAIRS - 1))
                        nc.any.tensor_copy(osb[:, 384 * nb:384 * nb + 384], pc)
                    nc.sync.dma_start(out_d[128 * qb:128 * qb + 128, :], osb[:])

            loop = tc.For_i(0, reps, 1) if reps > 1 else contextlib.nullcontext()
            # First-rep warmup runs once, outside the rep loop; inside the
            # loop the J=3 phase prefetches the NEXT rep's J=0 tiles, so
            # consecutive reps software-pipeline across the loop boundary.
            for p in range(PAIRS):
                emit_qk_n(p, 0)
            emit_v(range(0, 4))
            with loop:
                for J in range(NQ):
                    nxt = (J + 1) % NQ
                    last = J == NQ - 1
                    # One filler unit per block slot; q/k halves and 2-row v
                    # chunks are separated so the single mm_ps buffer never
                    # head-of-line blocks the PE queue within a slot.  k-half
                    # of pair p lands after block (p,1), past the last reader
                    # of kT[p][0]; at J=3 the v prefetch must wait for the
                    # final block (all J=3 A.V groups read vt[0:4]).
                    filler = [
                        lambda: emit_qk_half(0, nxt, 0) if last else (
                            emit_qk_half(0, nxt, 0),
                            emit_v(range(4 * nxt, 4 * nxt + 2))),
                        lambda: emit_qk_half(0, nxt, 1),
                        lambda: emit_qk_half(1, nxt, 0) if last else (
                            emit_qk_half(1, nxt, 0),
                            emit_v(range(4 * nxt + 2, 4 * nxt + 4))),
                        lambda: emit_qk_half(1, nxt, 1),
                        lambda: emit_qk_half(2, nxt, 0),
                        lambda: (emit_qk_half(2, nxt, 1),
                                 emit_v(range(4 * nxt, 4 * nxt + 4))
                                 if last else None),
                    ]
                    blocks = [(p, hh) for p in range(PAIRS) for hh in (0, 1)]
                    for bi, (p, hh) in enumerate(blocks):
                        emit_head_J(p, hh, J)
                        if last:
                            # all earlier superblocks' cproj woven into the
                            # ACT-bound final attention phase
                            emit_cproj(range(2 * bi, 2 * bi + 2))
                        filler[bi]()
                    if last:
                        emit_cproj(range(12, 16))

    nc.compile()
    return nc


def _host_inputs(x, c_attn_w, c_attn_b, c_proj_w, c_proj_b):
    """Slice/cast per-core inputs. Core c: batch c//2, heads 6*(c%2)..+6."""
    wq = c_attn_w[:, 0:D]
    wk = c_attn_w[:, D:2 * D]
    wv = c_attn_w[:, 2 * D:3 * D]
    bq = c_attn_b[0, 0:D]
    bk = c_attn_b[0, D:2 * D]
    bv = c_attn_b[0, 2 * D:3 * D]

    # S^T layout: rows = keys, cols = queries; keep keys <= query (0/1,
    # multiplied into exp(S^T) post-activation)
    mask = np.triu(np.ones((128, 128), dtype=np.float32)).astype(BF16)

    per_hg = []
    for hg in range(2):
        g0 = HPC * hg
        cs = slice(DH * g0, DH * (g0 + HPC))  # 384 columns of this head group
        wqk = np.concatenate([wq[:, cs], wk[:, cs]], axis=1)
        wqkb = np.stack(
            [np.concatenate([bq[cs], bk[cs]])[128 * m:128 * m + 128]
             for m in range(6)], axis=1).astype(np.float32)
        wva = np.zeros((D + 1, HPC * 65), dtype=np.float32)
        vbb = np.zeros((1, HPC * 65), dtype=np.float32)
        for j in range(HPC):
            wva[0:D, 65 * j:65 * j + 64] = wv[:, DH * (g0 + j):DH * (g0 + j + 1)]
            wva[D, 65 * j:65 * j + 64] = bv[DH * (g0 + j):DH * (g0 + j + 1)]
            wva[D, 65 * j + 64] = 1.0
            vbb[0, 65 * j:65 * j + 64] = bv[DH * (g0 + j):DH * (g0 + j + 1)]
            vbb[0, 65 * j + 64] = 1.0
        wp = c_proj_w[cs, :].astype(BF16)
        per_hg.append(dict(
            wqk=np.ascontiguousarray(wqk.astype(BF16)),
            wqkf8=np.ascontiguousarray((wqk * WS).astype(F8E4)),
            wqkb=np.ascontiguousarray(wqkb),
            wva=np.ascontiguousarray(wva.astype(BF16)),
            wp=np.ascontiguousarray(wp),
            mask=mask,
            vbb=np.ascontiguousarray(
                np.broadcast_to(vbb, (128, HPC * 65)).astype(BF16)),
        ))

    in_maps = []
    for c in range(NCORES):
        b, hg = divmod(c, 2)
        m = dict(per_hg[hg])
        xt = x[b].T
        m["xt"] = np.ascontiguousarray(xt.astype(BF16))
        m["xtf8"] = np.ascontiguousarray((xt * XS).astype(F8E4))
        in_maps.append(m)
    return in_maps


def _get_executor():
    """Build the program once and cache a jitted 8-core executor.

    Mirrors bass2jax.run_bass_via_pjrt's multi-core branch, but keeps the
    jitted function alive so repeat calls reuse the compiled executable.
    """
    if "exec" in _COMPILED:
        return _COMPILED["exec"]

    import jax
    import jax.numpy as jnp  # noqa: F401
    from jax.sharding import Mesh, PartitionSpec
    from jax.experimental.shard_map import shard_map
    import concourse.mybir as mybir
    from concourse import bass2jax

    nc = _build_program()
    bass2jax.install_neuronx_cc_hook()

    part_name = nc.partition_id_tensor.name if nc.partition_id_tensor else None
    in_names, out_names, out_avals, zero_outs = [], [], [], []
    for alloc in nc.m.functions[0].allocations:
        if not isinstance(alloc, mybir.MemoryLocationSet):
            continue
        name = alloc.memorylocations[0].name
        if alloc.kind == "ExternalInput":
            if name != part_name:
                in_names.append(name)
        elif alloc.kind == "ExternalOutput":
            out_names.append(name)
            shape = tuple(alloc.tensor_shape)
            dtype = mybir.dt.np(alloc.dtype)
            out_avals.append(jax.core.ShapedArray(shape, dtype))
            zero_outs.append(np.zeros(shape, dtype))
    n_params = len(in_names)
    n_outs = len(out_avals)
    all_names = in_names + out_names
    if part_name is not None:
        all_names = all_names + [part_name]
    donate = tuple(range(n_params, n_params + n_outs))

    def _body(*args):
        operands = list(args)
        if part_name is not None:
            operands.append(bass2jax.partition_id_tensor())
        outs = bass2jax._bass_exec_p.bind(
            *operands,
            out_avals=tuple(out_avals),
            in_names=tuple(all_names),
            out_names=tuple(out_names),
            lowering_input_output_aliases=(),
            sim_require_finite=True,
            sim_require_nnan=True,
            nc=nc,
        )
        return tuple(outs)

    devices = jax.devices()[:NCORES]
    mesh = Mesh(np.asarray(devices), ("core",))
    sharded = jax.jit(
        shard_map(
            _body, mesh=mesh,
            in_specs=(PartitionSpec("core"),) * (n_params + n_outs),
            out_specs=(PartitionSpec("core"),) * n_outs,
            check_rep=False,
        ),
        donate_argnums=donate, keep_unused=True,
    )

    def run(in_maps, device_out=False):
        concat_in = [
            np.concatenate([np.asarray(in_maps[c][nm]) for c in range(NCORES)],
                           axis=0)
            for nm in in_names
        ]
        concat_zeros = [
            np.zeros((NCORES * z.shape[0], *z.shape[1:]), z.dtype)
            for z in zero_outs
        ]
        out_arrs = sharded(*concat_in, *concat_zeros)
        if device_out:
            return out_arrs
        return [
            {nm: np.asarray(out_arrs[i]).reshape(NCORES, *out_avals[i].shape)[c]
             for i, nm in enumerate(out_names)}
            for c in range(NCORES)
        ]

    run.sharded = sharded
    run.in_names = in_names
    run.out_avals = out_avals
    run.zero_shapes = [
        ((NCORES * z.shape[0], *z.shape[1:]), z.dtype) for z in zero_outs
    ]
    _COMPILED["exec"] = run
    return run


def kernel(x, c_attn_w, c_attn_b, c_proj_w, c_proj_b):
    run = _get_executor()
    in_maps = _host_inputs(
        np.asarray(x), np.asarray(c_attn_w), np.asarray(c_attn_b),
        np.asarray(c_proj_w), np.asarray(c_proj_b))
    results = run(in_maps)

    out = np.empty((B, S, D), dtype=np.float32)
    bias = np.asarray(c_proj_b, dtype=np.float32).reshape(1, D)
    for b in range(B):
        out[b] = results[2 * b]["out"] + results[2 * b + 1]["out"] + bias
    return out
